# revision 43
# baseline (speedup 1.0000x reference)
"""Distributed multi-head attention kernel for 8 Trainium2 NeuronCores.

Problem: y = softmax((x Wq^T)(x Wk^T)^T / sqrt(D)) (x Wv^T) Wo^T + bo
with B=4, T=2048, C=1280, H=20, D=64, float32 I/O.

Sharding (sequence parallel, rank independent):
  Each core owns a T/8 token slice of all 4 batches (1024 tokens).
  It computes K/V projections per batch and AllGathers each batch's
  K^T and V as one bf16 payload (4 small AGs so comm starts and lands
  early); each batch's Q projection is emitted during its AG's
  flight. It then runs full attention for its queries over the
  gathered keys/values and applies the output projection for its
  tokens. The host reassembles the T axis.

Attention runs in a transposed "S_T[k, q]" layout. V is stored padded
per head as [.., 65] with a constant-1 column at index 64, so psum
row 64 of the P@V accumulation is sum_k P (the softmax denominator)
and rows 0..63 are the weighted values. This avoids partition-axis
reductions entirely. The inner loop works on 2-key-tile chunks with a
double-buffered score PSUM so the next chunk's QK^T matmuls never
wait for ScalarE to drain the previous chunk's exp. Both heads'
denominators share one batched reciprocal per head pair, and the
per-partition broadcast of the reciprocal goes through a DRAM-bounce
DMA (stride-0 read-back) so no compute engine spends cycles on it.
K and V loads are prefetched ahead (kp/vb bufs=4) to ride out
sync-queue head-of-line waits.

Compute dtype is bf16 (fp32 matmul is 4x slower on the PE array);
accumulation is fp32 in PSUM. I/O stays fp32.
"""

import os
import sys
import types

import numpy as np
import ml_dtypes

import concourse.bass as bass
import concourse.mybir as mybir
import concourse.tile as tile
from concourse import bacc
from concourse.bass_utils import run_bass_kernel_spmd

N_CORES = 8
C = 1280
H = 20
D = 64
B = 4
CT = C // 128  # 10 c-tiles
HP5 = H * 65   # padded V row width (ones column + 64 dims per head)
BF = mybir.dt.bfloat16
F32 = mybir.dt.float32
SCALE = 1.0 / (D ** 0.5)

LAST_EXEC_TIME_NS = None
_BUILD_CACHE = {}


def _install_ntff_hook():
    """The trimmed antenv package lacks axon_hooks; register the NTFF
    profile hook by hand so trace=True can time the NEFF on silicon.
    Safe no-op if anything is missing."""
    if "antenv.axon_hooks" in sys.modules:
        return
    try:
        from trn_agent_boot.trn_boot import _ntff_profile_via_ctypes

        hook = _ntff_profile_via_ctypes("/opt/axon/libaxon_pjrt.so")
        mod = types.ModuleType("antenv.axon_hooks")
        mod.get_axon_ntff_profile_hook = lambda: hook
        mod.set_axon_ntff_profile_hook = lambda h: None
        sys.modules["antenv.axon_hooks"] = mod
        import antenv

        antenv.axon_hooks = mod
    except Exception:
        pass


def _chunks(total, step):
    out = []
    o = 0
    while o < total:
        out.append((o, min(step, total - o)))
        o += step
    return out


def build(T):
    """Build the SPMD Bass graph for full (unsharded) sequence length T."""
    TS = T // N_CORES          # tokens per batch per core
    TOK = B * TS               # tokens per core
    KT = T // 128              # 128-wide key tiles per batch
    assert TS % 128 == 0, "key tiles must not cross rank chunks"
    assert TS <= 512, "q tile must fit one matmul moving operand"
    assert KT % 4 == 0
    NC2 = KT // 2              # 2-key-tile chunks per head per batch
    RG = [list(range(N_CORES))]

    nc = bacc.Bacc("TRN2", target_bir_lowering=False, debug=False,
                   num_devices=N_CORES)

    xT = nc.dram_tensor("xT", [C, TOK], BF, kind="ExternalInput").ap()
    wqT = nc.dram_tensor("wqT", [C, C], BF, kind="ExternalInput").ap()
    wkT = nc.dram_tensor("wkT", [C, C], BF, kind="ExternalInput").ap()
    wvT = nc.dram_tensor("wvT", [C, C], BF, kind="ExternalInput").ap()
    woT = nc.dram_tensor("woT", [C, C], BF, kind="ExternalInput").ap()
    bo_d = nc.dram_tensor("bo", [C, 1], F32, kind="ExternalInput").ap()
    out = nc.dram_tensor("out", [C, TOK], F32, kind="ExternalOutput").ap()

    with tile.TileContext(nc) as tc:
        with tc.tile_pool(name="dram", bufs=1, space="DRAM") as dram:
            # per-batch bounce + gathered buffers (K and V fused into one
            # AllGather payload per batch so comm starts/lands earlier)
            SZK = C * TS
            SZV = TS * HP5
            SZ = SZK + SZV
            kv_bn = [dram.tile([SZ], BF, name=f"kv_bn{i}") for i in range(B)]
            kv_all = [dram.tile([N_CORES * SZ], BF, addr_space="Shared",
                                name=f"kv_all{i}") for i in range(B)]
            k_bn = [t[0:SZK].rearrange("(r t) -> r t", t=TS) for t in kv_bn]
            v_bn = [t[SZK:SZ].rearrange("(p c) -> p c", c=HP5) for t in kv_bn]
            # bounce rows for the reciprocal partition-broadcast DMA
            rscr = [dram.tile([1, 2, TS], BF, name=f"rscr{i}")
                    for i in range(2)]

            with tc.tile_pool(name="persist", bufs=1) as persist:
                qT_sb = persist.tile([128, CT, TOK], BF)
                attn_sb = persist.tile([128, CT, TOK], BF)
                wo_sb = persist.tile([128, CT, C], BF)
                bo_sb = persist.tile([128, CT, 1], F32)

                def proj_T(psum, w_sb, dram_dst, sbuf_dst, t_lo, t_hi, pool,
                           dst_ofs=0):
                    # out[o, t] = sum_i W^T[i, o] x^T[i, t]
                    for ot in range(CT):
                        for t0, tsz in _chunks(t_hi - t_lo, 512):
                            t0 += t_lo
                            ps = psum.tile([128, 512], F32, tag="mm",
                                           bufs=6, name="ps_proj")
                            for i in range(CT):
                                nc.tensor.matmul(
                                    ps[:, :tsz],
                                    w_sb[:, i, ot * 128:(ot + 1) * 128],
                                    xT_sb[:, i, t0:t0 + tsz],
                                    start=(i == 0), stop=(i == CT - 1))
                            if sbuf_dst is not None:
                                nc.vector.tensor_copy(
                                    sbuf_dst[:, ot, t0:t0 + tsz],
                                    ps[:, :tsz])
                            else:
                                st = pool.tile([128, 512], BF, tag="st",
                                               bufs=4, name="st_proj")
                                nc.vector.tensor_copy(st[:, :tsz],
                                                      ps[:, :tsz])
                                nc.sync.dma_start(
                                    dram_dst[ot * 128:(ot + 1) * 128,
                                             t0 - dst_ofs:
                                             t0 - dst_ofs + tsz],
                                    st[:, :tsz])

                def proj_V(psum, b, pool):
                    # V (token-major, per-head 65-padded with a ones col
                    # at index 64 -> psum row 64 of P@V is the denominator)
                    for ttl in range(TS // 128):
                        tt = b * (TS // 128) + ttl
                        stv = pool.tile([128, H, 65], BF, tag="stv", bufs=2,
                                        name="stv")
                        nc.vector.memset(stv[:, :, 64:65], 1.0)
                        for o0, osz in _chunks(C, 512):
                            ps = psum.tile([128, 512], F32, tag="mm",
                                           bufs=6, name="ps_v")
                            for i in range(CT):
                                nc.tensor.matmul(
                                    ps[:, :osz],
                                    xT_sb[:, i, tt * 128:(tt + 1) * 128],
                                    wv_sb[:, i, o0:o0 + osz],
                                    start=(i == 0), stop=(i == CT - 1))
                            h0, nh = o0 // 64, osz // 64
                            nc.vector.tensor_copy(
                                stv[:, h0:h0 + nh, 0:64],
                                ps[:, :osz].rearrange("p (h c) -> p h c",
                                                      c=64))
                        nc.sync.dma_start(
                            v_bn[b][ttl * 128:(ttl + 1) * 128, :],
                            stv[:].rearrange("p h c -> p (h c)"))

                # ---------------- Phase 1: K/V projections + AGs ----------
                with tc.tile_pool(name="p1", bufs=1) as p1, \
                     tc.tile_pool(name="psum1", bufs=1, space="PSUM") as psum1:
                    xT_sb = p1.tile([128, CT, TOK], BF)
                    nc.sync.dma_start(
                        xT_sb[:], xT.rearrange("(n p) t -> p n t", p=128))
                    wk_sb = p1.tile([128, CT, C], BF)
                    nc.sync.dma_start(
                        wk_sb[:], wkT.rearrange("(n p) o -> p n o", p=128))
                    wv_sb = p1.tile([128, CT, C], BF)
                    nc.sync.dma_start(
                        wv_sb[:], wvT.rearrange("(n p) o -> p n o", p=128))
                    wq_sb = p1.tile([128, CT, C], BF)
                    nc.sync.dma_start(
                        wq_sb[:], wqT.rearrange("(n p) o -> p n o", p=128))

                    for b in range(B):
                        proj_T(psum1, wk_sb, k_bn[b], None,
                               b * TS, (b + 1) * TS, p1,
                               dst_ofs=b * TS)
                        proj_V(psum1, b, p1)
                        nc.gpsimd.collective_compute(
                            "AllGather", mybir.AluOpType.bypass,
                            replica_groups=RG,
                            ins=[kv_bn[b][:].opt()],
                            outs=[kv_all[b][:].opt()])
                        # this batch's Q^T lands while its AllGather flies,
                        # so batch-0 attention starts as soon as AG0 does
                        proj_T(psum1, wq_sb, None, qT_sb,
                               b * TS, (b + 1) * TS, p1)

                    nc.sync.dma_start(
                        wo_sb[:], woT.rearrange("(n p) o -> p n o", p=128))
                    nc.sync.dma_start(
                        bo_sb[:], bo_d.rearrange("(n p) o -> p n o", p=128))

                # ------------- Phase 2+3: attention + out-proj -------------
                with tc.tile_pool(name="p2", bufs=1) as p2, \
                     tc.tile_pool(name="psum2", bufs=1, space="PSUM") as psum2:
                    JR = TS // 128  # key tiles per rank chunk

                    for b in range(B):
                        kv_s = kv_all[b][:].rearrange(
                            "(s x) -> s x", s=N_CORES)
                        k_all_v = kv_s[:, 0:SZK].rearrange(
                            "s (r t) -> r s t", t=TS)
                        v_all_v = kv_s[:, SZK:SZ].rearrange(
                            "s (j p c) -> s j p c", p=128, c=HP5)

                        # batch 0's loads issue from the idle scalar queue:
                        # on the sync queue they would sit behind batches
                        # 1-3's projection-store waits (which only resolve
                        # as those projections finish) and delay the first
                        # exp by ~70us
                        ld = nc.scalar if b == 0 else nc.sync

                        # K^T for this batch's first head pair: prefetch
                        # before V so the first QK matmuls start earliest
                        kps = {}

                        def load_kp(hp):
                            kp = p2.tile([128, N_CORES, TS], BF, tag="kp",
                                         bufs=4, name="kp")
                            ld.dma_start(
                                kp[:],
                                k_all_v[hp * 128:(hp + 1) * 128, :, :])
                            return kp

                        kps[0] = load_kp(0)

                        # V for this batch, two halves of key tiles
                        vbs = []
                        for kh in range(2):
                            vb = p2.tile([128, KT // 2, HP5], BF, tag="vb",
                                         bufs=4, name=f"vb{kh}")
                            for s0 in range(N_CORES // 2):
                                s = kh * (N_CORES // 2) + s0
                                ld.dma_start(
                                    vb[:, s0 * JR:(s0 + 1) * JR, :],
                                    v_all_v[s, :, :, :]
                                    .rearrange("j p c -> p j c"))
                            vbs.append(vb)

                        for hp in range(CT):
                            kp = kps.pop(hp)
                            kp_f = kp[:].rearrange("p s t -> p (s t)")
                            pav0 = psum2.tile([65, TS], F32, tag="pav",
                                              bufs=2, name="pav0")
                            pav1 = psum2.tile([65, TS], F32, tag="pav",
                                              bufs=2, name="pav1")
                            pavs = (pav0, pav1)

                            for c2 in range(NC2):
                                ps = psum2.tile([128, 2, 2, TS], F32,
                                                tag="ps", bufs=2, name="ps")
                                for j in range(2):
                                    kt = c2 * 2 + j
                                    for h in range(2):
                                        nc.tensor.matmul(
                                            ps[:, h, j, :],
                                            kp_f[h * 64:(h + 1) * 64,
                                                 kt * 128:(kt + 1) * 128],
                                            qT_sb[h * 64:(h + 1) * 64, hp,
                                                  b * TS:(b + 1) * TS],
                                            start=True, stop=True,
                                            tile_position=(h * 64, 0))
                                P = p2.tile([128, 2, 2, TS], BF, tag="P",
                                            bufs=4, name="P")
                                nc.scalar.activation(
                                    P[:], ps[:],
                                    mybir.ActivationFunctionType.Exp,
                                    scale=SCALE)
                                kh, ltl = c2 // (NC2 // 2), c2 % (NC2 // 2)
                                for h in range(2):
                                    hg = 2 * hp + h
                                    for j in range(2):
                                        nc.tensor.matmul(
                                            pavs[h][:],
                                            vbs[kh][:, 2 * ltl + j,
                                                    hg * 65:(hg + 1) * 65],
                                            P[:, h, j, :],
                                            start=(c2 == 0 and j == 0),
                                            stop=(c2 == NC2 - 1 and j == 1))

                            # prefetch next head pair's keys
                            if hp + 1 < CT:
                                kps[hp + 1] = load_kp(hp + 1)

                            # softmax denominators for both heads: one
                            # reciprocal, then a partition-broadcast via a
                            # DRAM bounce (no PE or GpSimd involvement)
                            den = p2.tile([65, 2, TS], F32, tag="den",
                                          bufs=2, name="den")
                            nc.vector.tensor_copy(den[64:65, 0, :],
                                                  pav0[64:65, :])
                            nc.vector.tensor_copy(den[64:65, 1, :],
                                                  pav1[64:65, :])
                            # drain pav to SBUF promptly so the psum slots
                            # free for the next head pair
                            pav_sbs = []
                            for h in (0, 1):
                                pav_sb = p2.tile([64, TS], F32,
                                                 tag="pav_sb", bufs=3,
                                                 name="pav_sb")
                                nc.vector.tensor_copy(pav_sb[:],
                                                      pavs[h][0:64, :])
                                pav_sbs.append(pav_sb)
                            recip = p2.tile([65, 2, TS], BF, tag="recip",
                                            bufs=2, name="recip")
                            with nc.allow_low_precision(
                                    reason="softmax denom in bf16"):
                                nc.vector.reciprocal(recip[64:65, :, :],
                                                     den[64:65, :, :])
                            nc.sync.dma_start(rscr[hp % 2][:],
                                              recip[64:65, :, :])
                            bcast = p2.tile([64, 2, TS], BF, tag="bcast",
                                            bufs=2, name="bcast")
                            nc.sync.dma_start(
                                bcast[:],
                                rscr[hp % 2][:].broadcast_to([64, 2, TS]))
                            for h in (0, 1):
                                tmp = p2.tile([64, TS], BF, tag="tmp",
                                              bufs=3, name="tmp")
                                nc.vector.tensor_mul(tmp[:],
                                                     pav_sbs[h][:],
                                                     bcast[:, h, :])
                                nc.sync.dma_start(
                                    attn_sb[h * 64:(h + 1) * 64, hp,
                                            b * TS:(b + 1) * TS],
                                    tmp[:])

                        # out projection for this batch's tokens
                        for co in range(CT):
                            psy = psum2.tile([128, TS], F32, tag="psy",
                                             bufs=2, name="psy")
                            for ct in range(CT):
                                nc.tensor.matmul(
                                    psy[:],
                                    wo_sb[:, ct, co * 128:(co + 1) * 128],
                                    attn_sb[:, ct, b * TS:(b + 1) * TS],
                                    start=(ct == 0), stop=(ct == CT - 1))
                            ysb = p2.tile([128, TS], F32, tag="y", bufs=3,
                                          name="ysb")
                            nc.vector.tensor_scalar_add(
                                ysb[:], psy[:], bo_sb[:, co, :])
                            nc.sync.dma_start(
                                out[co * 128:(co + 1) * 128,
                                    b * TS:(b + 1) * TS],
                                ysb[:])

    nc.compile()
    return nc


def _prep_inputs(hidden_states, Wq, Wk, Wv, Wo, bo):
    T = hidden_states.shape[1]
    TS = T // N_CORES
    TOK = B * TS
    bf = ml_dtypes.bfloat16
    wqT = np.ascontiguousarray(np.asarray(Wq, np.float32).T).astype(bf)
    wkT = np.ascontiguousarray(np.asarray(Wk, np.float32).T).astype(bf)
    wvT = np.ascontiguousarray(np.asarray(Wv, np.float32).T).astype(bf)
    woT = np.ascontiguousarray(np.asarray(Wo, np.float32).T).astype(bf)
    bo_c = np.asarray(bo, np.float32).reshape(C, 1)
    x = np.asarray(hidden_states, np.float32)
    in_maps = []
    for r in range(N_CORES):
        xr = x[:, r * TS:(r + 1) * TS, :].reshape(TOK, C)
        xTr = np.ascontiguousarray(xr.T).astype(bf)
        in_maps.append({
            "xT": xTr, "wqT": wqT, "wkT": wkT, "wvT": wvT, "woT": woT,
            "bo": bo_c,
        })
    return in_maps


def kernel(hidden_states, Wq, Wk, Wv, Wo, bo):
    global LAST_EXEC_TIME_NS
    _install_ntff_hook()
    Bx, T, Cx = hidden_states.shape
    assert (Bx, Cx) == (B, C)
    TS = T // N_CORES
    if T not in _BUILD_CACHE:
        _BUILD_CACHE[T] = build(T)
    nc = _BUILD_CACHE[T]
    in_maps = _prep_inputs(hidden_states, Wq, Wk, Wv, Wo, bo)
    res = run_bass_kernel_spmd(nc, in_maps, core_ids=list(range(N_CORES)))
    LAST_EXEC_TIME_NS = res.exec_time_ns
    outf = np.empty((B, T, C), np.float32)
    for r in range(N_CORES):
        yT = res.results[r]["out"]          # [C, TOK]
        yr = yT.T.reshape(B, TS, C)
        outf[:, r * TS:(r + 1) * TS, :] = yr
    return outf


# revision 47
# speedup vs baseline: 1.1460x; 1.1460x over previous
"""Distributed multi-head attention kernel for 8 Trainium2 NeuronCores.

Problem: y = softmax((x Wq^T)(x Wk^T)^T / sqrt(D)) (x Wv^T) Wo^T + bo
with B=4, T=2048, C=1280, H=20, D=64, float32 I/O.

Sharding (sequence parallel, rank independent):
  Each core owns a T/8 token slice of all 4 batches (1024 tokens).
  It computes K/V projections per batch and AllGathers each batch's
  K^T and V as one bf16 payload (4 small AGs so comm starts and lands
  early); each batch's Q projection is emitted during its AG's
  flight. It then runs full attention for its queries over the
  gathered keys/values and applies the output projection for its
  tokens. The host reassembles the T axis.

Attention runs in a transposed "S_T[k, q]" layout. V is stored padded
per head as [.., 65] with a constant-1 column at index 64, so psum
row 64 of the P@V accumulation is sum_k P (the softmax denominator)
and rows 0..63 are the weighted values. This avoids partition-axis
reductions entirely. The inner loop works on 2-key-tile chunks with a
double-buffered score PSUM so the next chunk's QK^T matmuls never
wait for ScalarE to drain the previous chunk's exp. Both heads'
denominators share one batched reciprocal per head pair, and the
per-partition broadcast of the reciprocal goes through a DRAM-bounce
DMA (stride-0 read-back) so no compute engine spends cycles on it.
K and V loads are prefetched ahead (kp/vb bufs=4) to ride out
sync-queue head-of-line waits.

Compute dtype is bf16 (fp32 matmul is 4x slower on the PE array);
accumulation is fp32 in PSUM. I/O stays fp32.
"""

import os
import sys
import types

import numpy as np
import ml_dtypes

import concourse.bass as bass
import concourse.mybir as mybir
import concourse.tile as tile
from concourse import bacc
from concourse.bass_utils import run_bass_kernel_spmd

N_CORES = 8
C = 1280
H = 20
D = 64
B = 4
CT = C // 128  # 10 c-tiles
HP5 = H * 65   # padded V row width (ones column + 64 dims per head)
BF = mybir.dt.bfloat16
F32 = mybir.dt.float32
SCALE = 1.0 / (D ** 0.5)

LAST_EXEC_TIME_NS = None
_BUILD_CACHE = {}


def _install_ntff_hook():
    """The trimmed antenv package lacks axon_hooks; register the NTFF
    profile hook by hand so trace=True can time the NEFF on silicon.
    Safe no-op if anything is missing."""
    if "antenv.axon_hooks" in sys.modules:
        return
    try:
        from trn_agent_boot.trn_boot import _ntff_profile_via_ctypes

        hook = _ntff_profile_via_ctypes("/opt/axon/libaxon_pjrt.so")
        mod = types.ModuleType("antenv.axon_hooks")
        mod.get_axon_ntff_profile_hook = lambda: hook
        mod.set_axon_ntff_profile_hook = lambda h: None
        sys.modules["antenv.axon_hooks"] = mod
        import antenv

        antenv.axon_hooks = mod
    except Exception:
        pass


def _chunks(total, step):
    out = []
    o = 0
    while o < total:
        out.append((o, min(step, total - o)))
        o += step
    return out


def build(T):
    """Build the SPMD Bass graph for full (unsharded) sequence length T."""
    TS = T // N_CORES          # tokens per batch per core
    TOK = B * TS               # tokens per core
    KT = T // 128              # 128-wide key tiles per batch
    assert TS % 128 == 0, "key tiles must not cross rank chunks"
    assert TS <= 512, "q tile must fit one matmul moving operand"
    assert KT % 4 == 0
    NC2 = KT // 2              # 2-key-tile chunks per head per batch
    RG = [list(range(N_CORES))]

    nc = bacc.Bacc("TRN2", target_bir_lowering=False, debug=False,
                   num_devices=N_CORES)

    xT = nc.dram_tensor("xT", [C, TOK], BF, kind="ExternalInput").ap()
    wqT = nc.dram_tensor("wqT", [C, C], BF, kind="ExternalInput").ap()
    wkT = nc.dram_tensor("wkT", [C, C], BF, kind="ExternalInput").ap()
    wvT = nc.dram_tensor("wvT", [C, C], BF, kind="ExternalInput").ap()
    woT = nc.dram_tensor("woT", [C, C], BF, kind="ExternalInput").ap()
    bo_d = nc.dram_tensor("bo", [C, 1], F32, kind="ExternalInput").ap()
    out = nc.dram_tensor("out", [C, TOK], F32, kind="ExternalOutput").ap()

    with tile.TileContext(nc) as tc:
        with tc.tile_pool(name="dram", bufs=1, space="DRAM") as dram:
            # per-batch bounce + gathered buffers (K and V fused into one
            # AllGather payload per batch so comm starts/lands earlier)
            SZK = C * TS
            SZV = TS * HP5
            SZ = SZK + SZV
            kv_bn = [dram.tile([SZ], BF, name=f"kv_bn{i}") for i in range(B)]
            kv_all = [dram.tile([N_CORES * SZ], BF, addr_space="Shared",
                                name=f"kv_all{i}") for i in range(B)]
            k_bn = [t[0:SZK].rearrange("(r t) -> r t", t=TS) for t in kv_bn]
            v_bn = [t[SZK:SZ].rearrange("(p c) -> p c", c=HP5) for t in kv_bn]
            # bounce rows for the reciprocal partition-broadcast DMA
            rscr = [dram.tile([1, 2, TS], BF, name=f"rscr{i}")
                    for i in range(2)]

            with tc.tile_pool(name="persist", bufs=1) as persist:
                qT_sb = persist.tile([128, CT, TOK], BF)
                attn_sb = persist.tile([128, CT, TOK], BF)
                wo_sb = persist.tile([128, CT, C], BF)
                bo_sb = persist.tile([128, CT, 1], F32)

                def proj_T(psum, w_sb, dram_dst, sbuf_dst, t_lo, t_hi, pool,
                           dst_ofs=0):
                    # out[o, t] = sum_i W^T[i, o] x^T[i, t]
                    for ot in range(CT):
                        for t0, tsz in _chunks(t_hi - t_lo, 512):
                            t0 += t_lo
                            ps = psum.tile([128, 512], F32, tag="mm",
                                           bufs=6, name="ps_proj")
                            for i in range(CT):
                                nc.tensor.matmul(
                                    ps[:, :tsz],
                                    w_sb[:, i, ot * 128:(ot + 1) * 128],
                                    xT_sb[:, i, t0:t0 + tsz],
                                    start=(i == 0), stop=(i == CT - 1))
                            if sbuf_dst is not None:
                                nc.vector.tensor_copy(
                                    sbuf_dst[:, ot, t0:t0 + tsz],
                                    ps[:, :tsz])
                            else:
                                st = pool.tile([128, 512], BF, tag="st",
                                               bufs=4, name="st_proj")
                                nc.vector.tensor_copy(st[:, :tsz],
                                                      ps[:, :tsz])
                                nc.sync.dma_start(
                                    dram_dst[ot * 128:(ot + 1) * 128,
                                             t0 - dst_ofs:
                                             t0 - dst_ofs + tsz],
                                    st[:, :tsz])

                def proj_V(psum, b, pool):
                    # V (token-major, per-head 65-padded with a ones col
                    # at index 64 -> psum row 64 of P@V is the denominator)
                    for ttl in range(TS // 128):
                        tt = b * (TS // 128) + ttl
                        stv = pool.tile([128, H, 65], BF, tag="stv", bufs=2,
                                        name="stv")
                        nc.vector.memset(stv[:, :, 64:65], 1.0)
                        for o0, osz in _chunks(C, 512):
                            ps = psum.tile([128, 512], F32, tag="mm",
                                           bufs=6, name="ps_v")
                            for i in range(CT):
                                nc.tensor.matmul(
                                    ps[:, :osz],
                                    xT_sb[:, i, tt * 128:(tt + 1) * 128],
                                    wv_sb[:, i, o0:o0 + osz],
                                    start=(i == 0), stop=(i == CT - 1))
                            h0, nh = o0 // 64, osz // 64
                            nc.vector.tensor_copy(
                                stv[:, h0:h0 + nh, 0:64],
                                ps[:, :osz].rearrange("p (h c) -> p h c",
                                                      c=64))
                        nc.sync.dma_start(
                            v_bn[b][ttl * 128:(ttl + 1) * 128, :],
                            stv[:].rearrange("p h c -> p (h c)"))

                # ---------------- Phase 1: K/V projections + AGs ----------
                with tc.tile_pool(name="p1", bufs=1) as p1, \
                     tc.tile_pool(name="psum1", bufs=1, space="PSUM") as psum1:
                    xT_sb = p1.tile([128, CT, TOK], BF)
                    nc.sync.dma_start(
                        xT_sb[:], xT.rearrange("(n p) t -> p n t", p=128))
                    wk_sb = p1.tile([128, CT, C], BF)
                    nc.sync.dma_start(
                        wk_sb[:], wkT.rearrange("(n p) o -> p n o", p=128))
                    wv_sb = p1.tile([128, CT, C], BF)
                    nc.sync.dma_start(
                        wv_sb[:], wvT.rearrange("(n p) o -> p n o", p=128))
                    wq_sb = p1.tile([128, CT, C], BF)
                    nc.sync.dma_start(
                        wq_sb[:], wqT.rearrange("(n p) o -> p n o", p=128))

                    for b in range(B):
                        proj_T(psum1, wk_sb, k_bn[b], None,
                               b * TS, (b + 1) * TS, p1,
                               dst_ofs=b * TS)
                        proj_V(psum1, b, p1)
                        nc.gpsimd.collective_compute(
                            "AllGather", mybir.AluOpType.bypass,
                            replica_groups=RG,
                            ins=[kv_bn[b][:].opt()],
                            outs=[kv_all[b][:].opt()])
                        # early batches' Q^T lands while their AllGathers
                        # fly, so batch-0 attention starts as soon as AG0
                        # does; Q2/Q3 wait until after all K/V stores so
                        # those stores (which gate the sync queue's phase-2
                        # key loads) resolve sooner
                        if b < 2:
                            proj_T(psum1, wq_sb, None, qT_sb,
                                   b * TS, (b + 1) * TS, p1)

                    for b in (2, 3):
                        proj_T(psum1, wq_sb, None, qT_sb,
                               b * TS, (b + 1) * TS, p1)

                    nc.sync.dma_start(
                        wo_sb[:], woT.rearrange("(n p) o -> p n o", p=128))
                    nc.sync.dma_start(
                        bo_sb[:], bo_d.rearrange("(n p) o -> p n o", p=128))

                # ------------- Phase 2+3: attention + out-proj -------------
                with tc.tile_pool(name="p2", bufs=1) as p2, \
                     tc.tile_pool(name="psum2", bufs=1, space="PSUM") as psum2:
                    JR = TS // 128  # key tiles per rank chunk

                    for b in range(B):
                        kv_s = kv_all[b][:].rearrange(
                            "(s x) -> s x", s=N_CORES)
                        k_all_v = kv_s[:, 0:SZK].rearrange(
                            "s (r t) -> r s t", t=TS)
                        v_all_v = kv_s[:, SZK:SZ].rearrange(
                            "s (j p c) -> s j p c", p=128, c=HP5)

                        # K^T for this batch's first head pair: prefetch
                        # before V so the first QK matmuls start earliest
                        kps = {}

                        def load_kp(hp):
                            kp = p2.tile([128, N_CORES, TS], BF, tag="kp",
                                         bufs=4, name="kp")
                            nc.sync.dma_start(
                                kp[:],
                                k_all_v[hp * 128:(hp + 1) * 128, :, :])
                            return kp

                        # interleave key and value loads so early head
                        # pairs' keys are not queued behind all of V
                        def load_vb(kh):
                            vb = p2.tile([128, KT // 2, HP5], BF, tag="vb",
                                         bufs=4, name=f"vb{kh}")
                            for s0 in range(N_CORES // 2):
                                s = kh * (N_CORES // 2) + s0
                                nc.sync.dma_start(
                                    vb[:, s0 * JR:(s0 + 1) * JR, :],
                                    v_all_v[s, :, :, :]
                                    .rearrange("j p c -> p j c"))
                            return vb

                        kps[0] = load_kp(0)
                        vbs = [load_vb(0)]
                        kps[1] = load_kp(1)
                        vbs.append(load_vb(1))
                        kps[2] = load_kp(2)

                        for hp in range(CT):
                            kp = kps.pop(hp)
                            kp_f = kp[:].rearrange("p s t -> p (s t)")
                            pav0 = psum2.tile([65, TS], F32, tag="pav",
                                              bufs=2, name="pav0")
                            pav1 = psum2.tile([65, TS], F32, tag="pav",
                                              bufs=2, name="pav1")
                            pavs = (pav0, pav1)

                            for c2 in range(NC2):
                                ps = psum2.tile([128, 2, 2, TS], F32,
                                                tag="ps", bufs=2, name="ps")
                                for j in range(2):
                                    kt = c2 * 2 + j
                                    for h in range(2):
                                        nc.tensor.matmul(
                                            ps[:, h, j, :],
                                            kp_f[h * 64:(h + 1) * 64,
                                                 kt * 128:(kt + 1) * 128],
                                            qT_sb[h * 64:(h + 1) * 64, hp,
                                                  b * TS:(b + 1) * TS],
                                            start=True, stop=True,
                                            tile_position=(h * 64, 0))
                                P = p2.tile([128, 2, 2, TS], BF, tag="P",
                                            bufs=4, name="P")
                                nc.scalar.activation(
                                    P[:], ps[:],
                                    mybir.ActivationFunctionType.Exp,
                                    scale=SCALE)
                                kh, ltl = c2 // (NC2 // 2), c2 % (NC2 // 2)
                                for h in range(2):
                                    hg = 2 * hp + h
                                    for j in range(2):
                                        nc.tensor.matmul(
                                            pavs[h][:],
                                            vbs[kh][:, 2 * ltl + j,
                                                    hg * 65:(hg + 1) * 65],
                                            P[:, h, j, :],
                                            start=(c2 == 0 and j == 0),
                                            stop=(c2 == NC2 - 1 and j == 1))

                            # prefetch three head pairs ahead (0-2 are
                            # loaded at batch start)
                            if hp + 3 < CT:
                                kps[hp + 3] = load_kp(hp + 3)

                            # softmax denominators for both heads: one
                            # reciprocal, then a partition-broadcast via a
                            # DRAM bounce (no PE or GpSimd involvement)
                            den = p2.tile([65, 2, TS], F32, tag="den",
                                          bufs=2, name="den")
                            nc.vector.tensor_copy(den[64:65, 0, :],
                                                  pav0[64:65, :])
                            nc.vector.tensor_copy(den[64:65, 1, :],
                                                  pav1[64:65, :])
                            # drain pav to SBUF promptly so the psum slots
                            # free for the next head pair
                            pav_sbs = []
                            for h in (0, 1):
                                pav_sb = p2.tile([64, TS], F32,
                                                 tag="pav_sb", bufs=3,
                                                 name="pav_sb")
                                nc.vector.tensor_copy(pav_sb[:],
                                                      pavs[h][0:64, :])
                                pav_sbs.append(pav_sb)
                            recip = p2.tile([65, 2, TS], BF, tag="recip",
                                            bufs=2, name="recip")
                            with nc.allow_low_precision(
                                    reason="softmax denom in bf16"):
                                nc.vector.reciprocal(recip[64:65, :, :],
                                                     den[64:65, :, :])
                            nc.sync.dma_start(rscr[hp % 2][:],
                                              recip[64:65, :, :])
                            bcast = p2.tile([64, 2, TS], BF, tag="bcast",
                                            bufs=2, name="bcast")
                            nc.sync.dma_start(
                                bcast[:],
                                rscr[hp % 2][:].broadcast_to([64, 2, TS]))
                            for h in (0, 1):
                                tmp = p2.tile([64, TS], BF, tag="tmp",
                                              bufs=3, name="tmp")
                                nc.vector.tensor_mul(tmp[:],
                                                     pav_sbs[h][:],
                                                     bcast[:, h, :])
                                nc.sync.dma_start(
                                    attn_sb[h * 64:(h + 1) * 64, hp,
                                            b * TS:(b + 1) * TS],
                                    tmp[:])

                        # out projection for this batch's tokens
                        for co in range(CT):
                            psy = psum2.tile([128, TS], F32, tag="psy",
                                             bufs=2, name="psy")
                            for ct in range(CT):
                                nc.tensor.matmul(
                                    psy[:],
                                    wo_sb[:, ct, co * 128:(co + 1) * 128],
                                    attn_sb[:, ct, b * TS:(b + 1) * TS],
                                    start=(ct == 0), stop=(ct == CT - 1))
                            ysb = p2.tile([128, TS], F32, tag="y", bufs=3,
                                          name="ysb")
                            nc.vector.tensor_scalar_add(
                                ysb[:], psy[:], bo_sb[:, co, :])
                            nc.sync.dma_start(
                                out[co * 128:(co + 1) * 128,
                                    b * TS:(b + 1) * TS],
                                ysb[:])

    nc.compile()
    return nc


def _prep_inputs(hidden_states, Wq, Wk, Wv, Wo, bo):
    T = hidden_states.shape[1]
    TS = T // N_CORES
    TOK = B * TS
    bf = ml_dtypes.bfloat16
    wqT = np.ascontiguousarray(np.asarray(Wq, np.float32).T).astype(bf)
    wkT = np.ascontiguousarray(np.asarray(Wk, np.float32).T).astype(bf)
    wvT = np.ascontiguousarray(np.asarray(Wv, np.float32).T).astype(bf)
    woT = np.ascontiguousarray(np.asarray(Wo, np.float32).T).astype(bf)
    bo_c = np.asarray(bo, np.float32).reshape(C, 1)
    x = np.asarray(hidden_states, np.float32)
    in_maps = []
    for r in range(N_CORES):
        xr = x[:, r * TS:(r + 1) * TS, :].reshape(TOK, C)
        xTr = np.ascontiguousarray(xr.T).astype(bf)
        in_maps.append({
            "xT": xTr, "wqT": wqT, "wkT": wkT, "wvT": wvT, "woT": woT,
            "bo": bo_c,
        })
    return in_maps


def kernel(hidden_states, Wq, Wk, Wv, Wo, bo):
    global LAST_EXEC_TIME_NS
    _install_ntff_hook()
    Bx, T, Cx = hidden_states.shape
    assert (Bx, Cx) == (B, C)
    TS = T // N_CORES
    if T not in _BUILD_CACHE:
        _BUILD_CACHE[T] = build(T)
    nc = _BUILD_CACHE[T]
    in_maps = _prep_inputs(hidden_states, Wq, Wk, Wv, Wo, bo)
    res = run_bass_kernel_spmd(nc, in_maps, core_ids=list(range(N_CORES)))
    LAST_EXEC_TIME_NS = res.exec_time_ns
    outf = np.empty((B, T, C), np.float32)
    for r in range(N_CORES):
        yT = res.results[r]["out"]          # [C, TOK]
        yr = yT.T.reshape(B, TS, C)
        outf[:, r * TS:(r + 1) * TS, :] = yr
    return outf
